# revision 83
# baseline (speedup 1.0000x reference)
# Trainium2 Bass kernel for DirectSoftTreeEnsemble forward pass.
#
# Math (reference):
#   temp = clip(exp(log_temperature), 0.1, 5)
#   logits[b,t,i] = x[b,:] @ split_weights[t,i,:] + split_biases[t,i]
#   s = sigmoid(logits / temp)
#   mu[b,t,l]     = prod over path of s / (1-s)        (64 leaves, depth 6)
#   P[t,l,:]      = softmax(leaf_logits[t,l,:] / temp) (C=1000 classes)
#   w             = softmax(tree_weights)              (T=32 trees)
#   out[b,c]      = sum_{t,l} mu[b,t,l] * w[t] * P[t,l,c]
#
# Strategy: data-parallel over batch (4096 -> 8 cores x 512 rows), tree
# params replicated.  All x-independent math (leaf softmax, tree softmax,
# scale folding, sign folding, layout permutations) happens on host; the
# device runs two fp8-DR matmul stages with the sigmoid/doubling chain
# between them:
#   stage A: [512,1024] @ [1024,2048(ti)] -> sigmoid probs s (ACT),
#     kk-outer so each k-pair chunk of wT is consumed as it lands.
#   doubling: right = nu * s, then left = right - nu = (s-1) * nu --
#     two all-bf16 tensor_tensor ops per level (DVE 2x mode; the fused
#     scalar_tensor_tensor form has NO perf mode and runs 1x, so the
#     subtract form wins).  The sign flip is compensated in the
#     host-side dl rows, so (1-s) is never materialized.  Trees are
#     split DVE/Pool per m-tile (Pool is ~3.8x slower, small share).
#   transpose: PE transpose matmuls move mu6 into bf16 PSUM tiles
#     (53ns per 128x128 block at full clock, keeping the serial DMA
#     device free), then ACT/DVE evac them straight to the fp8 stage-B
#     lhsT -- the fp8 cast is fused into the evac, no separate pass.
#   stage B: [512,2048(tl)] @ [2048,1001] fp8 DR with the row-sum
#     correction column: dl col 1000 = sg*T*w_t so psum[:,1000]
#     recovers sum_t w_t*(sum_l mu) with the SAME fp8 mu errors,
#     cancelling them to first order.  ACT evacs psum with the
#     Identity(scale=GAMMA, bias=o1) fused form.
# dl[tl,c] = sg(l)*T*w_t*(C*P[tl,c]-1) in fp8 (delta-centered: ~10x
# better fp8 absolute error than raw P*C ~ 1.0); mu runs at 128x scale
# (folded into the level-1 init; e4m3 max finite is 240) and the final
# evac applies GAMMA = 1/(128*T*C).
#
# Cost-model notes (TimelineSim): matmul cost = out cols x 0.42ns x 0.5
# (fp8 DR) regardless of contraction depth; DMA is ONE serial device at
# ~360GB/s (elem>=512B); DVE gets 2x for all-bf16 tensor ops and the
# all-SBUF tensor_scalar; Pool cannot access PSUM; the PE p-state ramp
# (full clock only after 3us of continuous execution, reset by any idle
# gap) is held up by dummy-matmul fillers into about-to-be-reset psum
# regions during DMA/dependency waits.

import numpy as np
import ml_dtypes

import concourse.bass as bass
import concourse.mybir as mybir
import concourse.tile as tile
from concourse import bacc
from concourse.bass_utils import run_bass_kernel_spmd

BF16 = mybir.dt.bfloat16
F32 = mybir.dt.float32
FP8 = mybir.dt.float8e4
AF = mybir.ActivationFunctionType
OP = mybir.AluOpType
DR = mybir.MatmulPerfMode.DoubleRow

# Problem shapes (hardcoded per contract)
B, D, C, T, DEPTH = 4096, 1024, 1000, 32, 6
NI = 2**DEPTH - 1          # 63 internal nodes / tree
L = 2**DEPTH               # 64 leaves / tree
NIP = 64                   # padded internal nodes / tree
TIP = T * NIP              # 2048 padded internal total
TL = T * L                 # 2048 leaf rows total
NCORES = 8
BS = B // NCORES           # 512 batch rows / core
MT = BS // 128             # 4 m-tiles / core
KA = D // 128              # 8 k-tiles, stage A
KAP = KA // 2              # 4 k-pairs (DoubleRow), stage A
KB = TL // 128             # 16 k-tiles, stage B
KBP = KB // 2              # 8 pair-segs, stage B
DLW = 1008                 # dl row stride (16B-aligned, >=1001)
MUSCALE = 128.0            # mu pre-scale for fp8 range (e4m3 max finite 240)
GAMMA = 1.0 / (MUSCALE * T * C)
N_WARMUP_MM = 16

# doubling engine split: DVE takes trees [0, TSPLIT), Pool the rest
TSPLIT = 28


def _build(has_bias: bool, inv_temp: float):
    """Build the per-core SPMD Bass program."""
    nc = bacc.Bacc("TRN2", target_bir_lowering=False, debug=False)

    xT_d = nc.dram_tensor("xTh", [128, KA, BS], FP8, kind="ExternalInput")
    wT_d = nc.dram_tensor("wTh", [128, KA, TIP], FP8, kind="ExternalInput")
    dl_d = nc.dram_tensor("dl", [128, KB, DLW], FP8, kind="ExternalInput")
    id_d = nc.dram_tensor("identm", [128, 128], BF16, kind="ExternalInput")
    out_d = nc.dram_tensor("out", [BS, C], BF16, kind="ExternalOutput")
    if has_bias:
        bias_d = nc.dram_tensor("biasb", [128, TIP], F32, kind="ExternalInput")

    with tile.TileContext(nc) as tc:
        consts = tc.alloc_tile_pool(name="consts", bufs=1)
        work = tc.alloc_tile_pool(name="work", bufs=2)
        psp = tc.alloc_tile_pool(name="psp", bufs=4, space="PSUM")

        xTs = consts.tile([128, KA, BS], FP8)
        wTs = consts.tile([128, KA, TIP], FP8)
        dl = consts.tile([128, KB, DLW], FP8)
        identm = consts.tile([128, 128], BF16)
        muT8 = consts.tile([128, KB, BS], FP8)   # stage-B lhsT
        muT23 = consts.tile([128, KB, 128], BF16)  # m2 via DMA xbar

        # ---- input DMAs on the SP queue, few big chunks (each DMA costs
        # ~0.6us of issue/HWDGE time).  wT streams by (k-half, n-half):
        # the h0 tree half (1MB) lands first so sigmoids and doubling for
        # trees 0-15 start ~4us before the full wT is in ----
        nc.scalar.dma_start(xTs[:, 0:4, :], xT_d[:, 0:4, :])
        nc.scalar.dma_start(xTs[:, 4:8, :], xT_d[:, 4:8, :])
        nc.scalar.dma_start(identm, id_d[:, :])
        nc.sync.dma_start(wTs[:, 0:4, 0:1024], wT_d[:, 0:4, 0:1024])
        nc.sync.dma_start(wTs[:, 4:8, 0:1024], wT_d[:, 4:8, 0:1024])
        nc.sync.dma_start(wTs[:, 0:4, 1024:TIP], wT_d[:, 0:4, 1024:TIP])
        nc.sync.dma_start(wTs[:, 4:8, 1024:TIP], wT_d[:, 4:8, 1024:TIP])
        nc.sync.dma_start(dl[:, 0:4, :], dl_d[:, 0:4, :])
        nc.sync.dma_start(dl[:, 4:12, :], dl_d[:, 4:12, :])
        nc.sync.dma_start(dl[:, 12:16, :], dl_d[:, 12:16, :])
        if has_bias:
            biasb = consts.tile([128, TIP], F32)
            nc.sync.dma_start(biasb, bias_d[:, :])

        # PE warmup + fillers: dummy matmuls keep the PE p-state streak
        # alive through DMA waits (3us of continuous execution = full
        # clock; any idle gap resets to half speed).
        warm = consts.tile([128, 512], BF16)
        nc.gpsimd.memset(warm, 0.0)
        pwu = psp.tile([128, 1024], F32, name="pwu", tag="ps")

        def psum_fill(region, n):
            for _ in range(n):
                nc.tensor.matmul(region, warm[:, :128], warm[:, :128],
                                 start=True, stop=True)

        psum_fill(pwu[:, :128], N_WARMUP_MM)

        # ---- stage A: kk-outer per n-quarter (each psum accumulation
        # region closes as soon as its wT quarter is contracted, so the
        # sigmoid for 8 trees fires without waiting for the full wT) ----
        pa_t = {}

        def stage_a_q(ms, q, pre_fill=0):
            for kk in range(KAP):
                for m in ms:
                    msl = slice(m * 128, (m + 1) * 128)
                    if kk == 0 and q % 2 == 0:
                        pa_t[(m, q // 2)] = psp.tile(
                            [128, 1024], F32, name=f"pa{m}_{q // 2}",
                            tag="ps")
                        if pre_fill:
                            psum_fill(pa_t[(m, q // 2)][:, :128], pre_fill)
                            pre_fill = 0
                    dst = pa_t[(m, q // 2)][:, (q % 2) * 512:(q % 2 + 1) * 512]
                    nc.tensor.matmul(
                        dst, xTs[:, 2 * kk:2 * kk + 2, msl],
                        wTs[:, 2 * kk:2 * kk + 2, q * 512:(q + 1) * 512],
                        start=(kk == 0), stop=(kk == KAP - 1),
                        perf_mode=DR)

        def stage_a(ms, pre_fill=0):
            for q in range(4):
                stage_a_q(ms, q, pre_fill=pre_fill if q == 0 else 0)

        th_t = {}

        def sig_m(m, h):
            # s = sigmoid(z/temp) for trees [16h, 16h+16)
            if h == 0:
                th_t[m] = work.tile([128, TIP], BF16, name=f"th{m}",
                                    tag="th", bufs=4)
            pa = pa_t[(m, h)]
            hsl = slice(h * 1024, (h + 1) * 1024)
            if has_bias:
                nc.vector.tensor_tensor(pa, pa, biasb[:, hsl], OP.add)
            nc.scalar.activation(th_t[m][:, hsl], pa, AF.Sigmoid,
                                 scale=inv_temp)

        # ---- doubling ----
        mu6_t = {}

        def dbl_chain(m, t0, t1, eng, is_pool):
            th3 = th_t[m].rearrange("p (t i) -> p t i", t=T)
            nt = t1 - t0
            nuA = work.tile([128, nt * 32], BF16, name=f"nuA{m}_{t0}",
                            tag=f"nuA{t0}")
            nuB = work.tile([128, nt * 16], BF16, name=f"nuB{m}_{t0}",
                            tag=f"nuB{t0}")
            if m not in mu6_t:
                mu6_t[m] = work.tile([128, TL], BF16, name=f"mu6{m}",
                                     tag="mu6")
            mu6 = mu6_t[m].rearrange("p (t j) -> p t j", t=T)

            def lvl_view(d):
                buf = nuA if d % 2 == 1 else nuB
                return buf[:, :nt * (2 ** d)].rearrange(
                    "p (t j) -> p t j", t=nt)

            nu1 = lvl_view(1)
            eng.tensor_scalar(nu1[:, :, 0], th3[:, t0:t1, 1],
                              MUSCALE, -MUSCALE, OP.mult, OP.add)
            eng.tensor_scalar_mul(nu1[:, :, 1], th3[:, t0:t1, 1], MUSCALE)
            for d in range(1, DEPTH):
                lo, hi = 2 ** d, 2 ** (d + 1)
                nu_d = lvl_view(d)
                last = d == DEPTH - 1
                dst = mu6[:, t0:t1] if last else lvl_view(d + 1)
                half = 2 ** d
                # r = nu*s; l = r - nu = (s-1)*nu  (all-bf16 TT = DVE 2x;
                # STT has no perf mode so the subtract form is cheaper)
                eng.tensor_tensor(dst[:, :, half:], nu_d,
                                  th3[:, t0:t1, lo:hi], OP.mult)
                eng.tensor_tensor(dst[:, :, :half], dst[:, :, half:],
                                  nu_d, OP.subtract)

        # ---- mu transpose on PE (bf16 psum) + fused fp8 evac ----
        pt_t = {}

        def transpose_dma2(half):
            # m2's transpose rides the (idle-by-now) DMA xbar instead of
            # PE, with the fp8 cast in ACT's idle window
            c0 = half * 1024
            nc.sync.dma_start_transpose(
                muT23[:, 8 * half:8 * half + 8, :],
                mu6_t[2][:, c0:c0 + 1024])

        def cast2(half, eng):
            s0 = 8 * half
            if eng is nc.scalar:
                eng.activation(muT8[:, s0:s0 + 8, 2 * 128:3 * 128],
                               muT23[:, s0:s0 + 8, :], AF.Copy, scale=1.0)
            else:
                eng.tensor_scalar_mul(muT8[:, s0:s0 + 8, 2 * 128:3 * 128],
                                      muT23[:, s0:s0 + 8, :], 1.0)

        def transpose_m(m, half):
            pt = psp.tile([128, 8, 128], BF16, name=f"pt{m}_{half}",
                          tag="ps")
            pt_t[(m, half)] = pt
            for sq in range(8):
                s = 8 * half + sq
                nc.tensor.transpose(pt[:, sq, :],
                                    mu6_t[m][:, s * 128:(s + 1) * 128],
                                    identm)

        def evac_mut(m, half, eng):
            msl = slice(m * 128, (m + 1) * 128)
            pt = pt_t[(m, half)]
            s0 = 8 * half
            if eng is nc.scalar:
                eng.activation(muT8[:, s0:s0 + 8, msl], pt,
                               AF.Copy, scale=1.0)
            else:
                eng.tensor_scalar_mul(muT8[:, s0:s0 + 8, msl], pt, 1.0)


        # ---- stage B + output evac ----
        pb_t = {}

        def pb_fill(m, n):
            if m not in pb_t:
                pb_t[m] = psp.tile([128, 1024], F32, name=f"pb{m}", tag="ps")
            psum_fill(pb_t[m][:, :128], n)

        def stage_b(m, kk0, kk1):
            msl = slice(m * 128, (m + 1) * 128)
            if m not in pb_t:
                pb_t[m] = psp.tile([128, 1024], F32, name=f"pb{m}", tag="ps")
            for kk in range(kk0, kk1):
                k = 2 * kk
                for (c0, cn) in ((0, 512), (512, 489)):
                    nc.tensor.matmul(
                        pb_t[m][:, c0:c0 + cn], muT8[:, k:k + 2, msl],
                        dl[:, k:k + 2, c0:c0 + cn],
                        start=(kk == 0), stop=(kk == KBP - 1),
                        perf_mode=DR)

        def evac_out(m, eng, st_eng=None):
            msl = slice(m * 128, (m + 1) * 128)
            pb = pb_t[m]
            outm = work.tile([128, C], BF16, name=f"outm{m}", tag="outm")
            o1 = work.tile([128, 1], F32, name=f"o1{m}", tag="o1")
            nc.vector.tensor_scalar_mul(o1, pb[:, 1000:1001], GAMMA)
            if eng is nc.scalar:
                eng.activation(outm, pb[:, 0:1000], AF.Identity,
                               scale=GAMMA, bias=o1[:, :])
            else:
                eng.tensor_scalar(outm, pb[:, 0:1000],
                                  GAMMA, o1[:, :], OP.mult, OP.add)
            (st_eng or nc.sync).dma_start(out_d[msl, :], outm)

        # ---- emission order ----
        # PE: warm | A01-h0 | A01-h1 | A2 | A3 | B0..B3
        # ACT: sigs (8), cast m0, oev0, cast m2, oev1, oev2, oev3
        # DVE: dbl h0(m0,m1), h1(m0..m3), h0 shares late, cast m3 (+o1s)
        # Pool: dbl mid-shares m0..m3, cast m1
        # DMA: x+wT(h0) | wT(h1) | dl | T(m,h) xbar | stores
        PS0, PS1 = 16, 27   # DVE trees [0,PS0)+[PS0,PS1), Pool [PS1,32)
        stage_a_q((0, 1), 0)
        psum_fill(pwu[:, :128], 16)
        stage_a_q((0, 1), 1)
        sig_m(0, 0)
        sig_m(1, 0)
        psum_fill(pwu[:, :128], 18)
        stage_a_q((0, 1), 2)
        stage_a_q((0, 1), 3)
        sig_m(0, 1)
        sig_m(1, 1)
        dbl_chain(0, 0, PS0, nc.vector, False)
        dbl_chain(1, 0, PS0, nc.vector, False)
        stage_a((2,), pre_fill=10)
        sig_m(2, 0)
        sig_m(2, 1)
        stage_a((3,), pre_fill=4)
        sig_m(3, 0)
        sig_m(3, 1)
        dbl_chain(0, PS0, PS1, nc.vector, False)
        dbl_chain(1, PS0, PS1, nc.vector, False)
        dbl_chain(0, PS1, T, nc.gpsimd, True)
        dbl_chain(1, PS1, T, nc.gpsimd, True)
        transpose_m(0, 0)
        transpose_m(0, 1)
        evac_mut(0, 0, nc.scalar)
        evac_mut(0, 1, nc.scalar)
        dbl_chain(2, 0, PS0, nc.vector, False)
        dbl_chain(2, PS0, PS1, nc.vector, False)
        dbl_chain(2, PS1, T, nc.gpsimd, True)
        pb_fill(0, 24)
        stage_b(0, 0, 4)
        transpose_m(1, 0)
        transpose_m(1, 1)
        evac_mut(1, 0, nc.scalar)
        evac_mut(1, 1, nc.scalar)
        dbl_chain(3, 0, PS0, nc.vector, False)
        dbl_chain(3, PS0, PS1, nc.vector, False)
        dbl_chain(3, PS1, T, nc.gpsimd, True)
        pb_fill(1, 10)
        stage_b(0, 4, KBP)
        stage_b(1, 0, KBP)
        transpose_dma2(0)
        transpose_dma2(1)
        cast2(0, nc.scalar)
        cast2(1, nc.scalar)
        evac_out(0, nc.scalar)
        transpose_m(3, 0)
        transpose_m(3, 1)
        evac_out(1, nc.vector)
        evac_mut(3, 0, nc.vector)
        evac_mut(3, 1, nc.scalar)
        pb_fill(2, 8)
        stage_b(2, 0, KBP)
        pb_fill(3, 8)
        stage_b(3, 0, KBP)
        evac_out(2, nc.scalar)
        evac_out(3, nc.vector)

        psp.release()
        work.release()
        consts.release()

    nc.compile()
    return nc


_cache = {}


def _get_nc(key):
    if key not in _cache:
        _cache[key] = _build(*key)
    return _cache[key]


def kernel(x, split_weights, split_biases, leaf_logits, tree_weights,
           log_temperature):
    x = np.asarray(x, np.float32)
    split_weights = np.asarray(split_weights, np.float32)
    split_biases = np.asarray(split_biases, np.float32)
    leaf_logits = np.asarray(leaf_logits, np.float32)
    tree_weights = np.asarray(tree_weights, np.float32)
    lt = float(np.asarray(log_temperature, np.float32).reshape(-1)[0])

    has_bias = bool(np.any(split_biases != 0.0))
    temp = float(np.clip(np.exp(lt), 0.1, 5.0))
    f8 = ml_dtypes.float8_e4m3

    # ---- host layout prep ----
    # Node permutation: within each 64-col tree block, col 0 is padding and
    # level d occupies cols [2^d, 2^(d+1)) holding BFS node (2^d-1)+bitrev_d(r)
    # at col 2^d + r; leaves end up in LSB-first path order = bitrev6(BFS).
    def bitrev(v, bits):
        r = 0
        for _ in range(bits):
            r = (r << 1) | (v & 1)
            v >>= 1
        return r

    node_src = np.zeros(NIP, np.int64)  # padded col -> BFS node (col 0 -> pad)
    for d in range(DEPTH):
        for r in range(2 ** d):
            node_src[2 ** d + r] = (2 ** d - 1) + bitrev(r, d)
    leaf_src = np.array([bitrev(j, DEPTH) for j in range(L)], np.int64)
    # sign of position j: (-1)^(number of left steps) = (-1)^popcount(j)
    sg = np.array([(-1.0) ** bin(j).count("1") for j in range(L)], np.float64)

    # W^T [D, TIP] -> pre-tiled [128, KA, TIP], fp8
    wpad = np.zeros((T, NIP, D), np.float32)
    wpad[:, 1:, :] = split_weights[:, node_src[1:], :]
    wT = wpad.reshape(TIP, D).T  # [D, TIP]
    wTh = np.ascontiguousarray(
        wT.reshape(KA, 128, TIP).transpose(1, 0, 2).astype(f8))
    # x^T shards, pre-tiled [128, KA, BS] fp8
    xT = x.T.astype(f8)  # [D, B]
    xT_shards = []
    for cix in range(NCORES):
        sh = xT[:, cix * BS:(cix + 1) * BS]
        xT_shards.append(np.ascontiguousarray(
            sh.reshape(KA, 128, BS).transpose(1, 0, 2)))

    # leaf softmax + all folds on host (f64):
    #   delta[tl, c] = sg * T*w_t * (C*P - 1);  delta[tl, 1000] = sg * T*w_t
    # where tl indexes mu6 columns (tree-major, bitrev leaf positions).
    twf = tree_weights.astype(np.float64)
    twf = twf - twf.max()
    w = np.exp(twf) / np.exp(twf).sum()          # [T]
    ll = leaf_logits.astype(np.float64) / temp   # [T, L, C]
    ll = ll - ll.max(axis=-1, keepdims=True)
    P = np.exp(ll)
    P /= P.sum(axis=-1, keepdims=True)           # [T, L, C]
    P = P[:, leaf_src, :]                        # bitrev leaf order
    dlv = np.zeros((T, L, DLW), np.float64)
    dlv[:, :, :C] = (T * w[:, None, None]) * (C * P - 1.0)
    dlv[:, :, C] = T * w[:, None]
    dlv *= sg[None, :, None]
    dflat = dlv.reshape(TL, DLW)
    # dl row k*128+p pairs with muT8[p, k, :] = mu6 col k*128+p
    dl = np.ascontiguousarray(
        dflat.reshape(KB, 128, DLW).transpose(1, 0, 2).astype(f8))

    identm = np.eye(128, dtype=ml_dtypes.bfloat16)
    in_map_common = {"wTh": wTh, "dl": dl, "identm": identm}
    if has_bias:
        bpad = np.zeros((T, NIP), np.float32)
        bpad[:, 1:] = split_biases[:, node_src[1:]]
        in_map_common["biasb"] = np.ascontiguousarray(
            np.broadcast_to(bpad.reshape(1, TIP), (128, TIP)).astype(np.float32))

    nc = _get_nc((has_bias, 1.0 / temp))
    in_maps = [{"xTh": xT_shards[cix], **in_map_common}
               for cix in range(NCORES)]
    res = run_bass_kernel_spmd(nc, in_maps, core_ids=list(range(NCORES)))
    global LAST_RESULT
    LAST_RESULT = res
    out = np.concatenate([np.asarray(r["out"]).astype(np.float32)
                          for r in res.results], axis=0)
    return np.ascontiguousarray(out)


LAST_RESULT = None
